# revision 27
# baseline (speedup 1.0000x reference)
"""Trainium2 Bass kernel for nn_NeuralMMMModel (MMM: adstock scan + saturation + MLPs).

Key math: the reference's lax.scan over T only feeds its LAST carry downstream:
    last_ad[b, c] = sum_t d[c]^(T-1-t) * x[b, t, c],   d = sigmoid(decay) < 1.
Old timesteps decay geometrically, so steps whose weight falls below ~1e-8
contribute nothing representable in fp32; we truncate to the last K steps,
choosing K at runtime from the actual decay/alpha/|x| values (K == T when
decay is close to 1).

Device layout: channels on partitions (C=128), t-major free dim [half][t][b].
The weighted reduction over t runs SPLIT across two engines:
  - DVE: per-t fused multiply-accumulate  acc = x_t * d^(K-1-t) + acc
    (scalar_tensor_tensor, per-partition scalar = d-power column), ping-ponged
    across two accumulators so consecutive ops never RAW-chain;
  - PE: per-t accumulating matmuls with DIAGONAL lhsT Diag(d^(K-1-t)) into a
    PSUM bank (fp32), which also merges the DVE accumulators via a final
    identity-lhsT matmul, so ACT reads one finished PSUM tile.
This replaces a single-engine DVE tensor_tensor_scan, which ran at ~2.6-3.8
cycles/element and dominated the kernel (scan ~23-33us vs DMA ~13-15us).

The critical path is DMA-tail shaped: per-DMA overhead is ~0.75us, so half 0
ships as ONE chunk (its compute hides under half 1's DMA), while half 1
front-loads its DVE timesteps into the first chunk and ships its last
timesteps as small all-PE chunks, so the post-DMA path is a few matmuls.

The whole kernel uses ONE ACT table set (sigmoid_and_others: sigmoid, erf,
identity), so there are no mid-kernel ACT table reloads:
  - saturation: r = 1/sigmoid(bcl*last_ad) = 1 + exp(-bcl*last_ad), with the
    extra 1 folded into the next layer's bias on the host;
  - exact gelu via erf: 2*gelu(u) = u*(1+erf(u/sqrt2)), with the 0.5 folded
    into the next layer's weights on the host.
Epilogue biases b1 ride into PSUM via 1-deep matmuls against a ones-row, so
both W1-halves finish with a single wide GELU. The channel-interaction output
layer is folded on the host (interactions are never observed, so
W2 @ Wo1[:128] collapses the middle Linear), as is the control-vars Linear
(Wc @ Wo1[128:160]); the epilogue is then the minimal serial chain
exp -> mm -> gelu -> mm -> gelu -> mm -> copy -> DMA per half, and each
half's epilogue overlaps the other half's DMA + reduction. Dummy bf16
matmuls chained to each half's first chunk keep the PE HAM monitor warm so
the fp32 matmuls run at 2.4 GHz.

Sharding: pure data parallelism, batch B=2048 split across 8 cores (256 each).
"""

import contextlib
import numpy as np
from contextlib import ExitStack

import concourse.bass as bass
import concourse.tile as tile
from concourse import mybir, bacc
from concourse.bass_utils import run_bass_kernel_spmd

B, T, C, NCTRL = 2048, 512, 128, 10
NCORES = 8
BS = B // NCORES          # 256 batch rows per core
HALF = BS // 2            # 128 rows per half
HID = 2 * C               # 256
HO = 64

F32 = mybir.dt.float32
WARM = 2                  # immediate PE warm-up matmuls at body start
XBUFS = 2                 # x-tile buffers per chunk tag

_kernel_cache: dict[int, object] = {}


def _mix(ln, npe, flip):
    """Engine letters for one chunk: npe PE steps first (oldest, smallest
    weights), then DVE steps alternating between the A/B accumulators."""
    out = []
    for j in range(ln):
        if j < npe:
            out.append('P')
        else:
            out.append('A' if flip else 'B')
            flip = not flip
    return out, flip


def _plan(K: int):
    """Per-half chunk lists [(t0, ln, engines)] and the PE timestep list.

    Half 0: one big chunk (compute hides under half 1's DMA; fewer DMAs is
    cheaper than finer overlap). Half 1: DVE timesteps front-loaded into the
    first chunk, later timesteps all-PE so the post-DMA path is short.
    """
    if K <= 40:
        n_tail = min(12, max(2, K // 3))          # trailing all-PE steps
        k0 = K - n_tail
        flip = True
        m0, flip = _mix(K, int(round(K * 0.41)), flip)   # half 0: PE 14/34
        h0 = [(0, K, m0)]
        m1, flip = _mix(k0, max(0, k0 - 13), flip)       # half 1 c0: DVE 13
        h1 = [(0, k0, m1)]
        t0 = k0
        while t0 < K:
            ln = min(8, K - t0)
            if K - (t0 + ln) < 4 and K - (t0 + ln) > 0:
                ln = K - t0 - 4
            h1.append((t0, ln, ['P'] * ln))
            t0 += ln
    else:
        # Generic fallback for large K: symmetric chunks, mixed engines.
        flip = True
        halves = []
        for _ in range(2):
            ch = []
            t0 = 0
            while t0 < K:
                ln = min(24, K - t0)
                m, flip = _mix(ln, int(round(ln * 0.4)), flip)
                ch.append((t0, ln, m))
                t0 += ln
            halves.append(ch)
        h0, h1 = halves
    pe_ts = []
    for hp in (h0, h1):
        for t0, ln, m in hp:
            for j, e in enumerate(m):
                if e == 'P' and (t0 + j) not in pe_ts:
                    pe_ts.append(t0 + j)
    return [h0, h1], sorted(set(pe_ts))


def _par_layout(K: int):
    half_plans, pe_ts = _plan(K)
    npe = len(pe_ts)
    off = {}
    o = 0
    def take(name, w):
        nonlocal o
        off[name] = o
        o += w
    take("BCL", 1)            # [128, 1]  -max(beta, 0.01)
    take("W1N", 256)          # -(W1 * 2*sigmoid(alpha))
    take("W2OA", HO)          # W2[0:128] @ Wo1[:128]   (interactions folded)
    take("W2OB", HO)          # W2[128:256] @ Wo1[:128]
    take("WCOMBO", HO)        # rows 0:10 = Wc @ Wo1[128:160]
    take("WO2", 1)            # rows 0:64 = 0.5*Wo2[:, 0]
    take("B1PR", 256)         # row 0: b1 + 2*colsum(W1*a2), as 256 columns
    take("BO1P", 1)           # rows 0:64
    take("DPOW", K)           # col t = d^(K-1-t)
    take("DIAG", (npe + 1) * 128)  # Diag(d^(K-1-t)) per PE t, then identity
    return off, o, half_plans, pe_ts


def _build(K: int, reps: int = 1, mode: str = "full"):
    """Build + compile the Bass program for truncation length K.

    reps > 1 wraps the whole compute body in a hardware For_i loop
    (re-reading the same inputs); used only for steady-state HW timing."""
    OFF, PW, half_plans, pe_ts = _par_layout(K)
    npe = len(pe_ts)
    pe_block = {t: i for i, t in enumerate(pe_ts)}

    nc = bacc.Bacc("TRN2", target_bir_lowering=False, debug=False,
                   num_devices=NCORES)
    xt = nc.dram_tensor("xt", [C, 2 * K * HALF], F32, kind="ExternalInput")
    params = nc.dram_tensor("params", [128, PW], F32, kind="ExternalInput")
    cvt_in = nc.dram_tensor("cvt", [NCTRL, BS], F32, kind="ExternalInput")
    y_out = nc.dram_tensor("y", [1, BS], F32, kind="ExternalOutput")

    with tile.TileContext(nc) as tc, ExitStack() as ctx:
        const = ctx.enter_context(tc.tile_pool(name="const", bufs=1))
        xpools = {}
        for g, hp in enumerate(half_plans):
            for ci, (t0, ln, m) in enumerate(hp):
                xpools[(g, ci)] = ctx.enter_context(
                    tc.tile_pool(name=f"x{g}_{ci}", bufs=XBUFS))
        apool = ctx.enter_context(tc.tile_pool(name="acc", bufs=2))
        work = ctx.enter_context(tc.tile_pool(name="work", bufs=2))
        epool = ctx.enter_context(tc.tile_pool(name="epi", bufs=2))
        wpsum = ctx.enter_context(tc.tile_pool(name="wpsum", bufs=1, space="PSUM"))
        psum = ctx.enter_context(tc.tile_pool(name="psum", bufs=2, space="PSUM"))
        ephp = ctx.enter_context(tc.tile_pool(name="ephp", bufs=2, space="PSUM"))
        epop = ctx.enter_context(tc.tile_pool(name="epop", bufs=2, space="PSUM"))
        epyp = ctx.enter_context(tc.tile_pool(name="epyp", bufs=1, space="PSUM"))

        # Params go via SWDGE (gpsimd) so the x stream owns the sync HWDGE
        # queue from the first cycle.
        par = const.tile([128, PW], F32)
        nc.gpsimd.dma_start(out=par, in_=params[:, :])
        cvt = const.tile([128, BS], F32)
        nc.gpsimd.memset(cvt[:, :], 0.0)
        nc.gpsimd.dma_start(out=cvt[0:NCTRL, :], in_=cvt_in[:, :])
        ones = const.tile([1, HALF], F32)
        nc.gpsimd.memset(ones[:, :], 1.0)

        bcl = par[:, OFF["BCL"]:OFF["BCL"] + 1]
        warm_ps = wpsum.tile([1, 512], F32)
        parw = par[:, 0:512].bitcast(mybir.dt.bfloat16)

        def warm(src=None):
            s = parw if src is None else src
            nc.tensor.matmul(warm_ps[:, 0:512], lhsT=s[:, 0:1], rhs=s[:, 0:512])

        # staggered_reset: semaphore resets are staged instead of one
        # all-engine barrier, so adjacent iterations pipeline and the slope
        # measures steady-state throughput rather than body latency.
        with (tc.For_i(0, reps, 1, staggered_reset=True) if reps > 1
              else contextlib.nullcontext()):
         r = work.tile([128, BS], F32, tag="r", name="r")
         for _ in range(WARM):
             warm()

         for g, hp in enumerate(half_plans):
             accA = apool.tile([128, HALF], F32, tag="accA", name="accA")
             accB = apool.tile([128, HALF], F32, tag="accB", name="accB")
             ps = psum.tile([128, HALF], F32, tag="ps", name="ps")
             firstA = firstB = True
             first_pe = True
             has_dve = any(e != 'P' for _, _, m in hp for e in m)
             for ci, (t0, ln, mix) in enumerate(hp):
                 xg = xpools[(g, ci)].tile([128, ln * HALF], F32,
                                           tag=f"xg{g}_{ci}", name="xg")
                 nc.sync.dma_start(
                     out=xg,
                     in_=xt[:, (g * K + t0) * HALF:(g * K + t0 + ln) * HALF])
                 if mode == "dma":
                     continue
                 if ci == 0:
                     # PE warm-up chained to this half's first chunk, emitted
                     # BEFORE the diag group opens (in-group non-member
                     # matmuls corrupt the PSUM accumulation).
                     wsrc = xg[:, 0:256].bitcast(mybir.dt.bfloat16)
                     warm(wsrc)
                 for j, eng in enumerate(mix):
                     t = t0 + j
                     xi = xg[:, j * HALF:(j + 1) * HALF]
                     if eng == 'P':
                         blk = OFF["DIAG"] + pe_block[t] * 128
                         nc.tensor.matmul(ps, lhsT=par[:, blk:blk + 128],
                                          rhs=xi, start=first_pe, stop=False)
                         first_pe = False
                     else:
                         dcol = par[:, OFF["DPOW"] + t:OFF["DPOW"] + t + 1]
                         if eng == 'A':
                             if firstA:
                                 nc.vector.tensor_scalar_mul(
                                     out=accA, in0=xi, scalar1=dcol)
                                 firstA = False
                             else:
                                 nc.vector.scalar_tensor_tensor(
                                     out=accA, in0=xi, scalar=dcol, in1=accA,
                                     op0=mybir.AluOpType.mult,
                                     op1=mybir.AluOpType.add)
                         else:
                             if firstB:
                                 nc.vector.tensor_scalar_mul(
                                     out=accB, in0=xi, scalar1=dcol)
                                 firstB = False
                             else:
                                 nc.vector.scalar_tensor_tensor(
                                     out=accB, in0=xi, scalar=dcol, in1=accB,
                                     op0=mybir.AluOpType.mult,
                                     op1=mybir.AluOpType.add)

             if mode == "dma":
                 continue
             # Merge the DVE accumulators into PSUM: acc = accA + accB on
             # DVE, then one identity-lhsT matmul closes the PSUM group.
             if has_dve:
                 nc.vector.tensor_add(out=accA, in0=accA, in1=accB)
                 iblk = OFF["DIAG"] + npe * 128
                 nc.tensor.matmul(ps, lhsT=par[:, iblk:iblk + 128], rhs=accA,
                                  start=first_pe, stop=True)
             if mode == "phase1":
                 continue
             # Saturation: r = exp(-bcl * last_ad), read from PSUM.
             b0 = g * HALF
             nc.scalar.activation(
                 out=r[:, b0:b0 + HALF], in_=ps,
                 func=mybir.ActivationFunctionType.Exp, scale=bcl)

             # ---- epilogue for this half ----
             rh = r[:, b0:b0 + HALF]

             # h = 2*gelu(b1p - (W1*a2).T @ r): biases ride into PSUM via
             # 1-deep matmuls against a ones-row, one wide GELU finishes both
             # 128-column halves.
             hp2 = ephp.tile([128, 2 * HALF], F32, tag="hp", name="hp")
             obr = OFF["B1PR"]
             o1w = OFF["W1N"]
             nc.tensor.matmul(hp2[:, 0:HALF], lhsT=par[0:1, obr:obr + 128],
                              rhs=ones, start=True, stop=False)
             nc.tensor.matmul(hp2[:, 0:HALF], lhsT=par[:, o1w:o1w + 128],
                              rhs=rh, start=False, stop=True)
             nc.tensor.matmul(hp2[:, HALF:], lhsT=par[0:1, obr + 128:obr + 256],
                              rhs=ones, start=True, stop=False)
             nc.tensor.matmul(hp2[:, HALF:], lhsT=par[:, o1w + 128:o1w + 256],
                              rhs=rh, start=False, stop=True)
             h = epool.tile([128, 2 * HALF], F32, tag="h", name="h")
             nc.scalar.activation(out=h, in_=hp2,
                                  func=mybir.ActivationFunctionType.Gelu,
                                  bias=0.0)

             # o1 = gelu((W2 @ Wo1[:128]).T @ h + Wcombo.T @ cv + bo1p):
             # interactions are never observed, so W2 @ Wo1 is folded on the
             # host and the whole middle layer collapses into this chain.
             op = epop.tile([HO, HALF], F32, tag="op", name="op")
             oa = OFF["W2OA"]
             ob = OFF["W2OB"]
             ow = OFF["WCOMBO"]
             nc.tensor.matmul(op, lhsT=par[:, ow:ow + HO],
                              rhs=cvt[:, b0:b0 + HALF],
                              start=True, stop=False)
             nc.tensor.matmul(op, lhsT=par[:, oa:oa + HO], rhs=h[:, 0:HALF],
                              start=False, stop=False)
             nc.tensor.matmul(op, lhsT=par[:, ob:ob + HO], rhs=h[:, HALF:],
                              start=False, stop=True)
             o1 = epool.tile([HO, HALF], F32, tag="o1", name="o1")
             nc.scalar.activation(out=o1, in_=op,
                                  func=mybir.ActivationFunctionType.Gelu,
                                  bias=par[0:HO, OFF["BO1P"]:OFF["BO1P"] + 1])

             # y = (0.5*Wo2).T @ o1, 64-deep contraction (bo2 added on host)
             yp = epyp.tile([1, HALF], F32, tag="yp", name="yp")
             ow2 = OFF["WO2"]
             nc.tensor.matmul(yp, lhsT=par[0:HO, ow2:ow2 + 1], rhs=o1)
             ysb = epool.tile([1, HALF], F32, tag="ysb", name="ysb")
             nc.vector.tensor_copy(out=ysb, in_=yp)
             # y ships on the ACT HWDGE queue so the sync queue holds ONLY
             # the x stream.
             nc.scalar.dma_start(out=y_out[:, b0:b0 + HALF], in_=ysb)

         if mode in ("dma", "phase1"):
             nc.scalar.dma_start(out=y_out[:, :], in_=par[0:1, 0:BS])

    nc.compile()
    return nc


def _pick_K(d64, bcl64, maxabs):
    """Smallest K <= T whose truncated tail is < 3e-7 in z = bcl*last_ad."""
    d_max = float(d64.max())
    if d_max >= 1.0 - 1e-12:
        return T
    bcl_max = float(bcl64.max())
    scale = max(bcl_max * max(maxabs, 1e-30) / (1.0 - d_max), 1e-30)
    k = np.log(3e-7 / scale) / np.log(d_max)  # d_max^K * scale <= 3e-7
    return max(min(T, int(np.ceil(max(k, 1.0)))), 4)


def kernel(channel_spend, control_vars, decay, alpha, beta,
           W1, b1, W2, b2, Wc, bc, Wo1, bo1, Wo2, bo2):
    x = np.asarray(channel_spend, dtype=np.float32)
    cv = np.asarray(control_vars, dtype=np.float32)
    decay = np.asarray(decay, dtype=np.float64)
    alpha = np.asarray(alpha, dtype=np.float64)
    beta = np.asarray(beta, dtype=np.float64)
    W1 = np.asarray(W1, dtype=np.float64)
    b1 = np.asarray(b1, dtype=np.float64)
    W2 = np.asarray(W2, dtype=np.float32)
    b2 = np.asarray(b2, dtype=np.float64)
    Wc = np.asarray(Wc, dtype=np.float64)
    bc = np.asarray(bc, dtype=np.float64)
    Wo1 = np.asarray(Wo1, dtype=np.float64)
    bo1 = np.asarray(bo1, dtype=np.float64)
    Wo2 = np.asarray(Wo2, dtype=np.float32)
    bo2 = np.asarray(bo2, dtype=np.float64)

    d64 = 1.0 / (1.0 + np.exp(-decay))
    a64 = 2.0 / (1.0 + np.exp(-alpha))
    bcl64 = np.maximum(beta, 0.01)

    maxabs = max(abs(float(x.max())), abs(float(x.min())))
    K = _pick_K(d64, bcl64, maxabs)

    OFF, PW, half_plans, pe_ts = _par_layout(K)
    npe = len(pe_ts)

    W1a = W1 * a64[:, None]                       # [C, 2C]
    wcombo = (Wc @ Wo1[128:128 + 32]).astype(np.float32)     # [10, 64]
    # h_pre = b1 + colsum(W1a) - W1a.T @ e,  e = exp(-bcl*last_ad)
    b1p = (b1 + W1a.sum(axis=0)).astype(np.float32)          # [2C]
    bo1p = (bo1 + b2 @ Wo1[:128] + bc @ Wo1[128:128 + 32]).astype(np.float32)
    bo2f = float(bo2.reshape(-1)[0])

    par_base = np.zeros((128, PW), dtype=np.float32)
    W2o = (np.asarray(W2, np.float64) @ Wo1[:128]).astype(np.float32)  # [2C, 64]
    par_base[:, OFF["BCL"]] = (-bcl64).astype(np.float32)
    par_base[:, OFF["W1N"]:OFF["W1N"] + 256] = (-W1a).astype(np.float32)
    par_base[:, OFF["W2OA"]:OFF["W2OA"] + HO] = W2o[0:128]
    par_base[:, OFF["W2OB"]:OFF["W2OB"] + HO] = W2o[128:256]
    par_base[0:NCTRL, OFF["WCOMBO"]:OFF["WCOMBO"] + HO] = wcombo
    par_base[0:HO, OFF["WO2"]] = Wo2[:, 0]
    par_base[0, OFF["B1PR"]:OFF["B1PR"] + 256] = b1p
    par_base[0:HO, OFF["BO1P"]] = bo1p
    # d powers: col t = d^(K-1-t)
    dpow = (d64[:, None] ** (K - 1 - np.arange(K))[None, :]).astype(np.float32)
    par_base[:, OFF["DPOW"]:OFF["DPOW"] + K] = dpow
    # diag blocks for PE timesteps + identity merge block
    cidx = np.arange(128)
    for i, t in enumerate(pe_ts):
        par_base[cidx, OFF["DIAG"] + i * 128 + cidx] = dpow[:, t]
    par_base[cidx, OFF["DIAG"] + npe * 128 + cidx] = 1.0

    in_maps = []
    for i in range(NCORES):
        xs = x[i * BS:(i + 1) * BS, T - K:, :]            # [BS, K, C]
        xti = np.ascontiguousarray(
            xs.reshape(2, HALF, K, C).transpose(3, 0, 2, 1))  # [C, 2, K, HALF]
        cvt_i = np.ascontiguousarray(cv[i * BS:(i + 1) * BS, :].T)
        in_maps.append({"xt": xti.reshape(C, 2 * K * HALF),
                        "params": par_base, "cvt": cvt_i})

    nc = _kernel_cache.get(K)
    if nc is None:
        nc = _build(K)
        _kernel_cache[K] = nc

    res = run_bass_kernel_spmd(nc, in_maps, core_ids=list(range(NCORES)))
    y = np.concatenate([r["y"].reshape(-1) for r in res.results])
    return (y + np.float32(bo2f)).astype(np.float32)
